# revision 19
# baseline (speedup 1.0000x reference)
"""Trainium2 Bass kernel for nn_Circuit (batch quantum circuit simulation).

Math: circuit = M^{tensor 10} with M = (u1 x u1) @ u2 @ (u1 x u1) applied to
every base-4 digit of the 20-qubit state index, data-parallel over batch
(one item per core).

Strategy: 4 data-stationary (DS) bf16 matmul stages; each matmul uses a
128-col block of the STATE as the PE stationary and a gate matrix as the
moving operand, computing out = X_block^T @ G — which both applies the gate
(contracting the partition dim) and transposes (block cols -> partitions).
  S1: contract (c,d3,d4,d5) via realified complex gate      [1x state pass]
  S2: contract (d6,d7,d8)xI(d2a), c in free, 2-term accum   [2x]
  S3: contract (d9,d0,d1)xI(e3a), c in free, 2-term accum   [2x]
  S4: contract M(d2)xI32,        c in free, 2-term accum    [2x]
PE work: 7*16384 = 114688 rows bf16 (1 cyc/row) = 47.8us @ 2.4GHz warm.

Drains (PSUM->SBUF, DVE+ACT, 1 elem/cycle/lane since fp32 src) are the other
binding resource: all gate column orders / slab layouts are host-permuted so
every drain is ONE [128,1024] instruction whose PSUM-side inner runs are
>=8B-aligned pairs (full 8B PSUM cachelines) and whose SBUF-side runs are
>=16 contiguous bf16 — strided-inner drains measured up to 4x slower and
stalled the PE into HAM re-throttling (the original 79us).

The PE is an in-order FIFO, so program order = PE order; windows are packed
so each is PE-bound rather than drain-bound (next rep's s1_half(0) rides in
S4's window; s2_half(0) starts a few tiles into s1_half(1)).  Steady state:
49.6us/rep on the NTFF device timeline vs a 48.5us all-PE-bound floor.
"""
import numpy as np
import ml_dtypes

import concourse.bacc as bacc
import concourse.tile as tile
import concourse.mybir as mybir

F32 = mybir.dt.float32
BF16 = mybir.dt.bfloat16

NQ = 20
BATCH = 8
DIM = 2 ** NQ
N_CORES = 8

_NC_CACHE = {}


def _gates(u1_re, u1_im, u2_re, u2_im):
    u1 = np.asarray(u1_re, np.float64) + 1j * np.asarray(u1_im, np.float64)
    u2 = np.asarray(u2_re, np.float64) + 1j * np.asarray(u2_im, np.float64)
    A = np.kron(u1, u1)
    M = A @ u2 @ A
    W3 = np.kron(M, np.kron(M, M))
    return M, W3


def _permbits(A, axis, perm):
    """Permute the bit-order of an axis: perm[i] = source bit position (MSB
    order) for output bit i."""
    n = len(perm)
    sh = list(A.shape)
    k = axis
    view = A.reshape(sh[:k] + [2] * n + sh[k + 1:])
    axes = (list(range(k)) + [k + p for p in perm]
            + list(range(k + n, k + n + len(sh) - k - 1)))
    return view.transpose(axes).reshape(sh)


def build_weights_f64(u1_re, u1_im, u2_re, u2_im):
    """[128, 1664]: G1 | M2_0 | M2_1 | M3_0 | M3_1 | M4_0 | M4_1.

    Moving-gate column orders (host-permuted to make drains coalesce):
      j1 = c'*64 + (e3b*16+e4*4+e5)*2 + e3a
      j2 = c'*128 + e8b*64 + d2a*32 + e6*8 + e7*2 + e8a
      j3 = c'*128 + e3a*64 + e9*16 + e0*4 + e1
      j4 = c'*128 + e2*32 + e6*8 + e7*2 + e8a             (natural)
    Row orders:
      G1: c*64 + d345;  M2: d678*2 + d2a;
      M3: d0*32 + d1a*16 + d9*4 + d1b*2 + e3a;
      M4: d2b*64 + d2a*32 + (e6*8 + e7*2 + e8a)
    """
    M, W3 = _gates(u1_re, u1_im, u2_re, u2_im)
    I2, I32 = np.eye(2), np.eye(32)
    Wr, Wi = W3.real, W3.imag
    Mr, Mi = M.real, M.imag
    coef = {(0, 0): Wr, (0, 1): Wi, (1, 0): -Wi, (1, 1): Wr}
    coefM = {(0, 0): Mr, (0, 1): Mi, (1, 0): -Mi, (1, 1): Mr}

    # cols (c', e3a, e3b, e4, e5) -> (c', e3b, e4, e5, e3a)
    G1 = np.block([[Wr.T, Wi.T], [-Wi.T, Wr.T]])
    G1 = _permbits(G1, 1, [0, 2, 3, 4, 5, 6, 1])

    def half2(c, cp):
        # rows (d678, d2a), natural cols (e678, d2a') ->
        # cols [e8b, d2a', e6a,e6b,e7a,e7b,e8a]
        h = np.kron(coef[(c, cp)].T, I2)
        return _permbits(h, 1, [5, 6, 0, 1, 2, 3, 4])

    def half3(c, cp):
        # rows natural (d9a,d9b,d0a,d0b,d1a,d1b,e3a) ->
        #   (d0a,d0b,d1a,d9a,d9b,d1b,e3a)  [matches X2 lo7 order]
        # cols natural (e901, e3a') -> [e3a', e9a,e9b,e0a,e0b,e1a,e1b]
        h = np.kron(coef[(c, cp)].T, I2)
        h = _permbits(h, 0, [2, 3, 4, 0, 1, 5, 6])
        return _permbits(h, 1, [6, 0, 1, 2, 3, 4, 5])

    def half4(c, cp):
        # natural rows (d2a,d2b,r5) -> [d2b, d2a, r5]; cols (e2, r') natural
        h = np.kron(coefM[(c, cp)].T, I32)
        return _permbits(h, 0, [1, 0, 2, 3, 4, 5, 6])

    M2_0 = np.concatenate([half2(0, 0), half2(0, 1)], axis=1)
    M2_1 = np.concatenate([half2(1, 0), half2(1, 1)], axis=1)
    M3_0 = np.concatenate([half3(0, 0), half3(0, 1)], axis=1)
    M3_1 = np.concatenate([half3(1, 0), half3(1, 1)], axis=1)
    M4_0 = np.concatenate([half4(0, 0), half4(0, 1)], axis=1)
    M4_1 = np.concatenate([half4(1, 0), half4(1, 1)], axis=1)

    packed = np.concatenate([G1, M2_0, M2_1, M3_0, M3_1, M4_0, M4_1], axis=1)
    return np.ascontiguousarray(packed)


def build_weights(u1_re, u1_im, u2_re, u2_im):
    return build_weights_f64(u1_re, u1_im, u2_re, u2_im).astype(
        ml_dtypes.bfloat16)


def build_nc(repeat=1):
    nc = bacc.Bacc("TRN2", target_bir_lowering=False, debug=False,
                   num_devices=N_CORES)
    xin_d = nc.dram_tensor("xin", [128, 16384], BF16,
                           kind="ExternalInput").ap()
    wts_d = nc.dram_tensor("wts", [128, 1664], BF16,
                           kind="ExternalInput").ap()
    xout_d = nc.dram_tensor("xout", [128, 16384], F32,
                            kind="ExternalOutput").ap()

    dcnt = [0]

    with tile.TileContext(nc) as tc:
        with tc.tile_pool(name="sb", bufs=1) as sb, \
             tc.tile_pool(name="ps", bufs=4, space="PSUM") as ps:

            wt = sb.tile([128, 1664], BF16, tag="wt")
            nc.gpsimd.dma_start(wt[:], wts_d)

            X1 = sb.tile([128, 16384], BF16, tag="x1")
            X2 = sb.tile([128, 16384], BF16, tag="x2")
            X3 = sb.tile([128, 16384], BF16, tag="x3")
            X4 = sb.tile([128, 16384], BF16, tag="x4")
            O = sb.tile([128, 16384], F32, tag="o")

            for k in range(8):
                eng = nc.sync if k % 2 == 0 else nc.scalar
                eng.dma_start(X1[:, 2048 * k:2048 * (k + 1)],
                              xin_d[:, 2048 * k:2048 * (k + 1)])

            G1 = wt[:, 0:128]
            M2 = [wt[:, 128:384], wt[:, 384:640]]
            M3 = [wt[:, 640:896], wt[:, 896:1152]]
            M4 = [wt[:, 1152:1408], wt[:, 1408:1664]]

            def drain(dst, src, eng=None):
                if eng == "dve":
                    nc.vector.tensor_copy(dst, src)
                elif eng == "act":
                    nc.scalar.copy(dst, src)
                elif dcnt[0] % 4 == 3:
                    nc.vector.tensor_copy(dst, src)
                    dcnt[0] += 1
                else:
                    nc.scalar.copy(dst, src)
                    dcnt[0] += 1

            # Slab layouts (col13 = hi6*128 + lo7); stationaries are the
            # contiguous lo7 fields, drains scatter into them:
            # X2: lo7 = d0*32+d1a*16+d9*4+d1b*2+e3a; hi6 = d2b*32+cp*... —
            #     col = d2b*8192 + cp*4096 + h*128 + lo7 (h = e3b*16+e4*4+e5)
            # X3: lo7 = d2b*64+d2a*32+e6*8+e7*2+e8a; hi6 = e8b*32+e3b*16+e4*4+e5
            # X4: lo7 = e3a*64+e9*16+e0*4+e1;        hi6 = e8b*32+e3b*16+e4*4+e5
            X2s1 = X2[:].rearrange(
                "p (d2b cph s r) -> p d2b s cph r",
                d2b=2, cph=64, s=8, r=16)
            X3d = X3[:].rearrange(
                "p (cp e8b t ke d2b v) -> p t d2b cp ke e8b v",
                cp=2, e8b=2, t=8, ke=4, d2b=2, v=64)
            X3d2 = X3[:].rearrange(
                "p (cp e8b t ke d2b v) -> p t d2b cp ke e8b v",
                cp=2, e8b=2, t=8, ke=4, d2b=2, v=64)
            X4d = X4[:].rearrange(
                "p (cp t ke w) -> p t cp ke w", cp=2, t=16, ke=4, w=128)
            Od = O[:].rearrange(
                "p (cp t ke u) -> p t cp ke u", cp=2, t=16, ke=4, u=128)

            def s1_tile(d2b, s, eng=None):
                d0, d1a = s >> 1, s & 1
                pt = ps.tile([128, 1024], F32, tag="ps")
                for k in range(8):
                    d9, d1b = k >> 1, k & 1
                    b1 = d0 * 32 + (d1a * 2 + d1b) * 8 + d9 * 2 + d2b
                    nc.tensor.matmul(pt[:, k * 128:(k + 1) * 128],
                                     X1[:, b1 * 128:(b1 + 1) * 128],
                                     G1, start=True, stop=True)
                # psum col = d9*256 + d1b*128 + cph*2 + e3a
                ptv = pt[:].rearrange("p (d9 d1b cph e3a) -> "
                                      "p cph d9 d1b e3a",
                                      d9=4, d1b=2, cph=64, e3a=2)
                # One [128,1024] drain per tile: dst runs of 16 contiguous
                # bf16; src pairs (e3a) are 8B-aligned full PSUM lines.
                if eng is None:
                    eng = "dve" if (d2b * 8 + s) % 2 == 0 else "act"
                drain(X2s1[:, d2b, s], ptv, eng=eng)

            def s2_tile(d2b, y, eng=None):
                t = d2b * 8 + y
                pt = ps.tile([128, 1024], F32, tag="ps")
                for k in range(4):
                    h2 = 4 * y + k
                    for c in range(2):
                        stat = X2[:, d2b * 8192 + c * 4096 + h2 * 128:
                                  d2b * 8192 + c * 4096 + (h2 + 1) * 128]
                        nc.tensor.matmul(pt[:, k * 256:(k + 1) * 256],
                                         stat, M2[c],
                                         start=(c == 0), stop=(c == 1))
                ptv = pt[:].rearrange("p (ke cp e8b v) -> p cp ke e8b v",
                                      ke=4, cp=2, e8b=2, v=64)
                if eng is None:
                    eng = "dve" if t % 2 == 0 else "act"
                if eng == "dve":
                    drain(X3d2[:, y, d2b], ptv, eng="dve")
                else:
                    # ACT APs lower to TENSOR3D: the whole-tile dst view has
                    # 4 non-collapsible dims, so split per cp (3 dims each).
                    for cp in range(2):
                        drain(X3d[:, y, d2b, cp], ptv[:, cp], eng="act")

            def s3_tile(y, e8b):
                t = e8b * 8 + y
                pt = ps.tile([128, 1024], F32, tag="ps")
                for k in range(4):
                    B3 = 4 * t + k
                    for c in range(2):
                        stat = X3[:, c * 8192 + B3 * 128:
                                  c * 8192 + (B3 + 1) * 128]
                        nc.tensor.matmul(pt[:, k * 256:(k + 1) * 256],
                                         stat, M3[c],
                                         start=(c == 0), stop=(c == 1))
                ptv = pt[:].rearrange("p (ke cp w) -> p cp ke w",
                                      ke=4, cp=2, w=128)
                drain(X4d[:, t], ptv, eng=("dve" if t % 2 == 0 else "act"))

            def s4_tile(y, e8b):
                t = e8b * 8 + y
                pt = ps.tile([128, 1024], F32, tag="ps")
                for k in range(4):
                    B4 = 4 * t + k
                    for c in range(2):
                        stat = X4[:, c * 8192 + B4 * 128:
                                  c * 8192 + (B4 + 1) * 128]
                        nc.tensor.matmul(pt[:, k * 256:(k + 1) * 256],
                                         stat, M4[c],
                                         start=(c == 0), stop=(c == 1))
                ptv = pt[:].rearrange("p (ke cp u) -> p cp ke u",
                                      ke=4, cp=2, u=128)
                drain(Od[:, t], ptv, eng=("dve" if t % 2 == 1 else "act"))

            # Window packing (PE is in-order FIFO, so program order = PE
            # order): every window should be PE-bound, not drain-bound.
            # - next rep's s1_half(0) rides inside S4's window (its MMs
            #   depend only on resident X1, never on fresh drains);
            # - s2_half(0) starts a few tiles into s1_half(1) so its X2
            #   d2b=0 drain dependency is hidden behind s1 MMs.
            for s in range(8):
                s1_tile(0, s)          # rep 0 prologue
            for _rep in range(repeat):
                s1_tile(1, 0)
                s1_tile(1, 1)
                for i in range(6):
                    s1_tile(1, i + 2)
                    s2_tile(0, i)
                s2_tile(0, 6)
                s2_tile(0, 7)
                for y in range(8):
                    s2_tile(1, y)
                for y in range(8):
                    for e8b in range(2):
                        s3_tile(y, e8b)
                last = _rep + 1 >= repeat
                for y in range(8):
                    s4_tile(y, 0)
                    s4_tile(y, 1)
                    if not last:
                        s1_tile(0, y)  # next rep's first S1 half
                for q in range(8):
                    eng = nc.sync if q % 2 == 0 else nc.gpsimd
                    eng.dma_start(xout_d[:, 2048 * q:2048 * (q + 1)],
                                  O[:, 2048 * q:2048 * (q + 1)])

    nc.compile()
    return nc


def _get_nc():
    if "nc" not in _NC_CACHE:
        _NC_CACHE["nc"] = build_nc()
    return _NC_CACHE["nc"]


def pack_state(x_real, x_imag, b):
    """[DIM] re/im planes of batch item b -> X1 [128, 16384] bf16.
    p1 = c*64 + d3*16 + d4*4 + d5
    f1 = (d0*32+d1*8+d9*2+d2b)*128 + (d6*32+d7*8+d8*2+d2a)."""
    a = np.stack([np.asarray(x_real[b], np.float32),
                  np.asarray(x_imag[b], np.float32)])
    a = a.reshape(2, 4, 4, 2, 2, 4, 4, 4, 4, 4, 4, 4)
    # axes: c d0 d1 d2a d2b d3 d4 d5 d6 d7 d8 d9
    a = a.transpose(0, 5, 6, 7, 1, 2, 11, 4, 8, 9, 10, 3)
    return np.ascontiguousarray(a.reshape(128, 16384)).astype(
        ml_dtypes.bfloat16)


def unpack_out(xo):
    """xout [B, 128, 16384] f32 -> [2, B, DIM].
    p5 = e3a*64 + e9*16 + e0*4 + e1
    f5 = c*8192 + (e8b*32+e3b*16+e4*4+e5)*128 + (e2*32+e6*8+e7*2+e8a)."""
    a = xo.reshape(-1, 2, 4, 4, 4,         # B e3a e9 e0 e1
                   2, 2, 2, 4, 4,          # c e8b e3b e4 e5
                   4, 4, 4, 2)             # e2 e6 e7 e8a
    # target: c, B, e0, e1, e2, e3a, e3b, e4, e5, e6, e7, e8a, e8b, e9
    a = a.transpose(5, 0, 3, 4, 10, 1, 7, 8, 9, 11, 12, 13, 6, 2)
    return np.ascontiguousarray(a.reshape(2, -1, DIM))


def make_runner(nc, n_cores=N_CORES):
    """Persistent sharded-jit callable for the compiled module."""
    import jax
    from jax.sharding import Mesh, PartitionSpec
    from jax.experimental.shard_map import shard_map
    import concourse.mybir as mybir_
    from concourse.bass2jax import (_bass_exec_p, install_neuronx_cc_hook,
                                    partition_id_tensor)

    install_neuronx_cc_hook()
    part_name = (nc.partition_id_tensor.name
                 if nc.partition_id_tensor is not None else None)
    in_names, out_names, out_avals, zero_outs = [], [], [], []
    for alloc in nc.m.functions[0].allocations:
        if not isinstance(alloc, mybir_.MemoryLocationSet):
            continue
        name = alloc.memorylocations[0].name
        if alloc.kind == "ExternalInput":
            if name != part_name:
                in_names.append(name)
        elif alloc.kind == "ExternalOutput":
            shape = tuple(alloc.tensor_shape)
            dtype = mybir_.dt.np(alloc.dtype)
            out_names.append(name)
            out_avals.append(jax.core.ShapedArray(shape, dtype))
            zero_outs.append(np.zeros(shape, dtype))
    n_params = len(in_names)
    all_names = in_names + out_names
    if part_name is not None:
        all_names = all_names + [part_name]

    def _body(*args):
        operands = list(args)
        if part_name is not None:
            operands.append(partition_id_tensor())
        outs = _bass_exec_p.bind(
            *operands,
            out_avals=tuple(out_avals),
            in_names=tuple(all_names),
            out_names=tuple(out_names),
            lowering_input_output_aliases=(),
            sim_require_finite=True,
            sim_require_nnan=True,
            nc=nc,
        )
        return tuple(outs)

    devices = jax.devices()[:n_cores]
    mesh = Mesh(np.asarray(devices), ("core",))
    specs = (PartitionSpec("core"),) * (n_params + len(out_names))
    out_specs = (PartitionSpec("core"),) * len(out_names)
    fn = jax.jit(shard_map(_body, mesh=mesh, in_specs=specs,
                           out_specs=out_specs, check_rep=False),
                 keep_unused=True)

    def run(in_maps=None, concat_args=None):
        args = []
        if concat_args is not None:
            for name in in_names:
                args.append(np.asarray(concat_args[name]))
        else:
            for name in in_names:
                args.append(np.concatenate(
                    [np.asarray(m[name]) for m in in_maps], axis=0))
        for z in zero_outs:
            args.append(np.zeros((n_cores * z.shape[0], *z.shape[1:]),
                                 z.dtype))
        outs = fn(*args)
        return {name: np.asarray(outs[i]) for i, name in enumerate(out_names)}

    return run


def _get_runner():
    if "run" not in _NC_CACHE:
        _NC_CACHE["run"] = make_runner(_get_nc())
    return _NC_CACHE["run"]


def kernel(x_real, x_imag, u1_re, u1_im, u2_re, u2_im):
    run = _get_runner()
    wts = build_weights(u1_re, u1_im, u2_re, u2_im)

    # pack all 8 cores in one vectorized op
    A = np.stack([np.asarray(x_real, np.float32),
                  np.asarray(x_imag, np.float32)], axis=1)     # [8, 2, DIM]
    A = A.reshape(BATCH, 2, 4, 4, 2, 2, 4, 4, 4, 4, 4, 4, 4)
    A = A.transpose(0, 1, 6, 7, 8, 2, 3, 12, 5, 9, 10, 11, 4)
    xin_all = np.ascontiguousarray(
        A.reshape(BATCH * 128, 16384)).astype(ml_dtypes.bfloat16)
    wts_all = np.tile(wts, (BATCH, 1))                         # [8*128, 1664]

    results = run(concat_args={"xin": xin_all, "wts": wts_all})
    xo = results["xout"].reshape(BATCH, 128, 16384)
    out = unpack_out(xo)
    return np.ascontiguousarray(out.astype(np.float32))

